# revision 53
# baseline (speedup 1.0000x reference)
"""Trainium2 Bass kernel for nn_ChannelAttentionLayer.

Reference computation (NCHW, x:(4,256,64,64)):
  Q = BN(conv3x3(x, Wq, pad=1))            -> (4,256,64,64)
  K = BN(conv1x1(x, Wk, pad=1))            -> (4,256,66,66)
  V = BN(conv1x1(x, Wv, pad=1))            -> (4,256,66,66)
  S = K^T Q over channels                  -> (4,4356,4096)
  attn = softmax(S, axis=keys)
  out = V @ attn                           -> (4,256,4096) -> (4,256,64,64)

Sharding: 8 cores = 4 batches x 2 query-halves.  Each core computes the
3x3 Q-conv for its 2048 query positions, and full-batch 1x1 K/V convs on
the 4096 *interior* positions only.

Key algebra (exact, not approximate):
 * Conv biases cancel under batch-stats BN -> dropped.
 * With K_bn = ak*K_raw + ck and Q_bn = aq*Q_raw + cq (per-channel BN
   affines), the score splits as
     s(k,q) = s0(k,q) + alpha(k) + beta(q) + gamma,
     s0 = K_raw^T ((ak*aq) * Q_raw),  alpha = K_raw^T (ak*cq).
   beta(q) and gamma are constant over keys, so they cancel in softmax.
 * Pad-ring key tokens have K_raw = V_raw = 0, so their scores equal
   exactly beta+gamma; their softmax weight is e^0 against e^~60 for
   real keys -> negligible (~1e-11), and their V contribution enters
   only via the exact `+cv * sum(attn) = +cv` epilogue term.  Keys
   therefore reduce to the 4096 interior tokens = exactly 32 tiles.
 * exp(alpha(k)) is folded into the rows of V^T (including the ones
   column used for the softmax denominator), so the exp eviction uses a
   single broadcast bias, and Q/K need no normalization passes at all:
   Q gets one scale-only pass (by aq*ak), K is used raw.
 * V and K sums via conv linearity: sum_pos V = Wv @ colsum(x) and
   sum_pos K = Wk @ colsum(x); V sumsq via a ones-matmul over (V^T)^2,
   deferred a few tiles behind the conv so PE never waits on the
   evict+square chain.

Per-core inputs (SPMD-uniform): `xq` = the 34 padded-grid rows of my
query half + halo (Q conv reads contiguous shifted spans, wrap columns
dropped at eviction); `xi` = the 4096 interior tokens pre-gathered by
the host in a fixed order (query rows first), so K/V convs and the
colsum use plain contiguous access patterns on every core.

All matmuls run as float32r (~tf32, full PE rate); softmax uses a fixed
shift (scores peak ~95).  BatchNorm batch-stats come from two small
AllReduces as in the reference (training mode).
"""
import math

import numpy as np

import concourse.bass as bass
import concourse.mybir as mybir
import concourse.tile as tile
from concourse.bass_utils import run_bass_kernel_spmd

dt = mybir.dt
AF = mybir.ActivationFunctionType
ALU = mybir.AluOpType
F32 = dt.float32
F32R = dt.float32r

N_CORES = 8
CT = 2                   # channel tiles (256 = 2 x 128)
H = W = 64
HP = 66                  # padded grid width
NPOS = H * W             # 4096 interior positions
NPAD = HP * HP           # 4356 padded positions
NKT = 32                 # key tiles: interior only, 32*128 = 4096
QSH = 2048               # query positions per core
CSHIFT = 104.0           # softmax shift; global max (s0+alpha) is ~95
EPS = 1e-5
NQ_TOT = float(4 * NPOS)
NKV_TOT2 = float(2 * 4 * NPAD)   # x2: both half-cores contribute full-batch sums


def _kt_row(kt):
    # first layout row of key tile kt (2 rows x 64 cols per tile)
    return 1 + 2 * kt if kt < 16 else 2 + 2 * kt


def _kb_row(b):
    # first layout row of K-conv block b (8 rows x 64 cols per block)
    return 1 + 8 * b if b < 4 else 2 + 8 * b


# ---------------------------------------------------------------------------
# Workaround: this walrus build rejects >1 semaphore wait per instruction.
# After Tile scheduling, move excess waits onto same-engine NoOp carriers
# inserted right before the over-subscribed instruction.
_UID = [0]


def _split_waits_in_module(nc):
    for fn in nc.m.functions:
        for blk in fn.blocks:
            insts = list(blk.instructions)
            if not any(
                i.sync_info and i.sync_info.on_wait and len(i.sync_info.on_wait) > 1
                for i in insts
            ):
                continue
            new = []
            for inst in insts:
                si = inst.sync_info
                waits = list(si.on_wait) if (si and si.on_wait) else []
                if len(waits) > 1:
                    for w in waits[:-1]:
                        _UID[0] += 1
                        new.append(
                            mybir.InstNoOp(
                                name=f"I-waitsplit-{_UID[0]}",
                                engine=inst.engine,
                                ins=[],
                                outs=[],
                                sync_info=mybir.SyncInfo(on_wait=[w], on_update=[]),
                            )
                        )
                    inst.sync_info = mybir.SyncInfo(
                        on_wait=waits[-1:], on_update=list(si.on_update or [])
                    )
                new.append(inst)
            del blk.instructions[:]
            for i in new:
                blk.instructions.append(i)


class TC(tile.TileContext):
    def __exit__(self, exc_type, exc_val, exc_tb):
        r = super().__exit__(exc_type, exc_val, exc_tb)
        if exc_type is None:
            _split_waits_in_module(self.nc)
        return r


# ---------------------------------------------------------------------------
def build_nc(reps: int = 1, skip_cc: bool = False):
    nc = bass.Bass("TRN2", target_bir_lowering=False, num_devices=N_CORES)

    xq_d = nc.dram_tensor("xq", [128, CT, 34 * HP], F32R, kind="ExternalInput")
    xi_d = nc.dram_tensor("xi", [128, CT, NPOS], F32R, kind="ExternalInput")
    wq_d = nc.dram_tensor("wq", [128, 9, CT, 256], F32R, kind="ExternalInput")
    wk_d = nc.dram_tensor("wk", [128, CT, 256], F32R, kind="ExternalInput")
    wv_d = nc.dram_tensor("wv", [128, CT, 256], F32R, kind="ExternalInput")
    vec_d = nc.dram_tensor("vecs", [128, 12], F32, kind="ExternalInput")
    y_d = nc.dram_tensor("y", [16, 128, 256], F32, kind="ExternalOutput")

    cc_in1 = nc.dram_tensor("cc_in1", [128, 8], F32)
    cc_out1 = nc.dram_tensor("cc_out1", [128, 8], F32, addr_space="Shared")
    cc_in2 = nc.dram_tensor("cc_in2", [128, 4], F32)
    cc_out2 = nc.dram_tensor("cc_out2", [128, 4], F32, addr_space="Shared")
    scr_ab = nc.dram_tensor("scr_ab", [512], F32)
    scr_v = nc.dram_tensor("scr_v", [512], F32)
    scr_k = nc.dram_tensor("scr_k", [256], F32)

    with TC(nc) as tc:
        with (
            tc.tile_pool(name="sb_in", bufs=1) as sb_in,
            tc.tile_pool(name="sb_w", bufs=1) as sb_w,
            tc.tile_pool(name="sb_small", bufs=1) as sb_small,
            tc.tile_pool(name="sb_tmp", bufs=2) as sb_tmp,
            tc.tile_pool(name="sb_e", bufs=4) as sb_e,
            tc.tile_pool(name="sb_out", bufs=4) as sb_out,
            tc.tile_pool(name="ps_s2", bufs=2, space="PSUM") as ps_s2,
            tc.tile_pool(name="psb1", bufs=4, space="PSUM") as psb1,
        ):
            def body(_it):
                f = F32

                # ------- loads, chunked so PE can start ASAP -------
                xq = sb_in.tile([128, CT, 34 * HP], F32R, tag="xq")
                xi = sb_in.tile([128, CT, NPOS], F32R, tag="xi")
                wv = sb_in.tile([128, CT, 256], F32R, tag="wv")
                wq = sb_in.tile([128, 9, CT, 256], F32R, tag="wq")
                wk = sb_in.tile([128, CT, 256], F32R, tag="wk")
                vecs = sb_in.tile([128, 12], f, tag="vecs")
                # single SP queue: ACT-queue DMA issues would delay the
                # early PSUM evictions queued behind them
                q0, q1 = nc.sync, nc.sync
                q0.dma_start(out=wv, in_=wv_d.ap())
                for ci in range(CT):
                    q0.dma_start(out=xi[:, ci, 0:640], in_=xi_d.ap()[:, ci, 0:640])
                for ci in range(CT):
                    q0.dma_start(out=xq[:, ci, 0:9 * HP],
                                 in_=xq_d.ap()[:, ci, 0:9 * HP])
                q0.dma_start(out=wq[:, :, :, 0:128], in_=wq_d.ap()[:, :, :, 0:128])
                q0.dma_start(out=wq[:, :, :, 128:256],
                             in_=wq_d.ap()[:, :, :, 128:256])
                for lo, hi in [(9 * HP, 16 * HP), (16 * HP, 24 * HP),
                               (24 * HP, 34 * HP)]:
                    for ci in range(CT):
                        q0.dma_start(out=xq[:, ci, lo:hi],
                                     in_=xq_d.ap()[:, ci, lo:hi])
                for ci in range(CT):
                    q0.dma_start(out=xi[:, ci, 640:NPOS],
                                 in_=xi_d.ap()[:, ci, 640:NPOS])
                q0.dma_start(out=wk, in_=wk_d.ap())
                q0.dma_start(out=vecs, in_=vec_d.ap())

                consts = sb_small.tile([128, 2], f, tag="consts")
                nc.vector.memset(consts[:, 0:1], EPS)
                nc.vector.memset(consts[:, 1:2], -CSHIFT)
                eps_t = consts[:, 0:1]
                negc_t = consts[:, 1:2]
                invn4 = sb_small.tile([128, 4], f, tag="invn4")
                nc.vector.memset(invn4[:, 0:2], 1.0 / NQ_TOT)
                nc.vector.memset(invn4[:, 2:4], 1.0 / NKV_TOT2)
                ones_r = sb_small.tile([128, 1], F32R, tag="ones_r")
                nc.vector.tensor_copy(
                    out=ones_r, in_=nc.const_aps.tensor(1.0, (128, 1), F32)
                )

                qraw = sb_w.tile([128, CT, QSH], F32R, tag="qraw")
                kraw = sb_w.tile([128, CT, NPOS], F32R, tag="kraw")
                pdum = ps_s2.tile([1, 512], f, tag="s2", name="pdum")
                for i in range(8):
                    nc.tensor.matmul(
                        pdum, kraw[:, 0, 0:1], kraw[:, 0, 0:512],
                        start=True, stop=True, skip_group_check=True,
                    )
                vt = sb_w.tile([128, NKT, 258], F32R, tag="vt")
                nc.vector.tensor_copy(
                    out=vt[:, :, 256:257],
                    in_=nc.const_aps.tensor(1.0, (128, NKT, 1), F32),
                )
                nc.vector.tensor_copy(
                    out=vt[:, :, 257:258],
                    in_=nc.const_aps.tensor(0.0, (128, NKT, 1), F32),
                )
                # stat partials: per mt: Qsum 0:2, Qsq 2:4, Ksum 4:8, Ksq 8:12
                qk_part = sb_small.tile([128, CT, 16], f, tag="qk_part")
                sums = sb_small.tile([128, 8], f, tag="sums")
                sums_v = sb_small.tile([128, 4], f, tag="sums_v")
                sqscr = sb_tmp.tile([128, 1024], f, tag="sq", bufs=1)
                sqscr2 = sb_tmp.tile([128, 1024], f, tag="sq2", bufs=1)
                sqscr3 = sb_tmp.tile([128, 1024], f, tag="sq3", bufs=1)

                # V sumsq accumulator psum, held across the whole conv phase
                pvs = psb1.tile([1, 512], f, tag="b1", name="pvs")

                # ------- V^T conv on interior tokens (tile kt = 2 rows x 64)
                # The per-tile sumsq matmul is deferred a few tiles behind the
                # conv so PE never waits on a tile's evict+square chain.
                vt_pend = []

                def flush_vt(n):
                    while len(vt_pend) > n:
                        kt, vt2 = vt_pend.pop(0)
                        nc.tensor.matmul(
                            pvs[0:1, 256:512], ones_r, vt2,
                            start=(kt == 0), stop=(kt == NKT - 1),
                        )

                def emit_vt(kt, borrow=False):
                    # evict/square engines balanced so no single engine paces
                    # the PE; Pool squares only while it is idle (V head)
                    pool = ps_s2 if borrow else psb1
                    pvt = pool.tile([128, 256], f, tag="s2" if borrow else "b1",
                                    name=f"pvt{kt}")
                    for ci in range(CT):
                        nc.tensor.matmul(
                            pvt,
                            xi[:, ci, kt * 128:(kt + 1) * 128],
                            wv[:, ci, :],
                            start=(ci == 0), stop=(ci == CT - 1),
                        )
                    vt2 = sb_tmp.tile([128, 256], F32R, tag="vt2", bufs=5,
                                      name=f"vt2_{kt}")
                    if kt % 2 == 0:
                        nc.scalar.activation(out=vt[:, kt, 0:256], in_=pvt,
                                             func=AF.Copy)
                        nc.gpsimd.tensor_mul(vt2, vt[:, kt, 0:256],
                                             vt[:, kt, 0:256])
                    else:
                        nc.vector.tensor_copy(out=vt[:, kt, 0:256], in_=pvt)
                        nc.scalar.activation(out=vt2, in_=vt[:, kt, 0:256],
                                             func=AF.Square)
                    vt_pend.append((kt, vt2))
                    flush_vt(2 if kt < 10 else 3)

                for kt in range(5):
                    emit_vt(kt)

                # ------- Q conv: 3x3 via 9 shifted contiguous spans of the
                # padded halo; wrap-garbage columns dropped at eviction
                QBLK = [(1, 7), (8, 7), (15, 7), (22, 7), (29, 4)]

                def emit_q(mt, bi):
                    r0, nr = QBLK[bi]
                    n = nr * HP - 2
                    pq = ps_s2.tile([128, 1024], f, tag="s2", name=f"pq{mt}{bi}")
                    first = True
                    for tap in range(9):
                        ty, tx = tap // 3, tap % 3
                        sft = (r0 + ty - 1) * HP + tx
                        for ci in range(CT):
                            nc.tensor.matmul(
                                pq[:, 0:n],
                                wq[:, tap, ci, mt * 128:(mt + 1) * 128],
                                xq[:, ci, sft:sft + n],
                                start=first, stop=(tap == 8 and ci == CT - 1),
                            )
                            first = False
                    qsl = qraw[:, mt, (r0 - 1) * 64:(r0 - 1 + nr) * 64]
                    nc.scalar.activation(
                        out=qsl.rearrange("p (a b) -> p a b", a=nr),
                        in_=pq[:, 0:nr * HP]
                        .rearrange("p (a b) -> p a b", a=nr)[:, :, 0:64],
                        func=AF.Copy, accum_out=qk_part[:, mt, bi:bi + 1],
                    )
                    nc.scalar.activation(
                        out=sqscr[:, 0:nr * 64], in_=qsl, func=AF.Square,
                        accum_out=qk_part[:, mt, 5 + bi:6 + bi],
                    )

                for mt in range(2):
                    for bi in range(5):
                        emit_q(mt, bi)

                # colsum(x) over interior rows, on DVE (feeds the V-mean
                # matmul much later; pad cols are zero so full rows are fine)
                colsum2 = sb_small.tile([128, 2], F32R, tag="colsum2")
                with nc.allow_low_precision(reason="f32r output is f32-width"):
                    for ci in range(CT):
                        nc.vector.reduce_sum(
                            out=colsum2[:, ci:ci + 1],
                            in_=xi[:, ci, :], axis=mybir.AxisListType.X,
                        )

                # V and K sums via conv linearity: sum_tok V = Wv@colsum(x),
                # sum_tok K = Wk@colsum(x).  Freeing the K evictions from the
                # accumulate keeps them off the ACT critical path, and the
                # K-sum column bounce runs long before the AllReduce needs it.
                pvm = psb1.tile([1, 512], f, tag="b1", name="pvm")
                for ci in range(CT):
                    nc.tensor.matmul(
                        pvm[0:1, 0:256], colsum2[:, ci:ci + 1], wv[:, ci, :],
                        start=(ci == 0), stop=(ci == CT - 1),
                    )
                for ci in range(CT):
                    nc.tensor.matmul(
                        pvm[0:1, 256:512], colsum2[:, ci:ci + 1], wk[:, ci, :],
                        start=(ci == 0), stop=(ci == CT - 1),
                    )
                vrow = sb_small.tile([1, 512], f, tag="vrow")
                krow = sb_small.tile([1, 256], f, tag="krow")
                nc.scalar.activation(out=vrow[0:1, 0:256], in_=pvm[0:1, 0:256],
                                     func=AF.Copy)
                nc.scalar.activation(out=krow, in_=pvm[0:1, 256:512], func=AF.Copy)
                nc.sync.dma_start(out=scr_k.ap(), in_=krow)
                nc.sync.dma_start(
                    out=sums[:, 2:4],
                    in_=bass.AP(tensor=scr_k, offset=0, ap=[[1, 128], [128, 2]]),
                )

                # ------- K conv on interior tokens: blocks of 2x(8 rows x 64)
                # ~19us of evict+sumsq work against ~7us of PE work: spread
                # over all three non-PE engines so none of them paces PE, and
                # emit all evictions before any sumsq so the evicts (which
                # gate PSUM reuse) never queue behind a sumsq.
                KE_ENG = [0, 1, 0, 1, 0, 1, 0, 1]     # evict: 0=ACT 1=DVE
                KS_ENG = [1, 0, 1, 0, 1, 0, 1, 0]     # ksq:   0=ACT 1=DVE

                def emit_k(mt, bp):
                    t = bp * 2 + mt
                    pk = ps_s2.tile([128, 1024], f, tag="s2", name=f"pk{mt}{bp}")
                    for half in range(2):
                        b = bp * 2 + half
                        for ci in range(CT):
                            nc.tensor.matmul(
                                pk[:, half * 512:(half + 1) * 512],
                                wk[:, ci, mt * 128:(mt + 1) * 128],
                                xi[:, ci, b * 512:(b + 1) * 512],
                                start=(ci == 0), stop=(ci == CT - 1),
                            )
                    ksl = kraw[:, mt, bp * 1024:(bp + 1) * 1024]
                    if KE_ENG[t] == 0:
                        nc.scalar.activation(out=ksl, in_=pk, func=AF.Copy)
                    else:
                        nc.vector.tensor_copy(out=ksl, in_=pk)

                def emit_ksq(mt, bp):
                    t = bp * 2 + mt
                    ksl = kraw[:, mt, bp * 1024:(bp + 1) * 1024]
                    part = qk_part[:, mt, 10 + bp:11 + bp]
                    if KS_ENG[t] == 0:
                        nc.scalar.activation(out=sqscr3, in_=ksl, func=AF.Square,
                                             accum_out=part)
                    else:
                        nc.vector.scalar_tensor_tensor(
                            out=sqscr2, in0=ksl, scalar=1.0, in1=ksl,
                            op0=ALU.mult, op1=ALU.mult,
                            accum_out=part,
                        )

                for bp in range(4):
                    for mt in range(2):
                        emit_k(mt, bp)
                for bp in range(4):
                    for mt in range(2):
                        emit_ksq(mt, bp)

                # V tiles resume: their evictions give the PE headroom
                # while the stats pipeline drains.
                for kt in range(5, 14):
                    emit_vt(kt)

                # ------- Q/K stats gather + AllReduce #1 launch.  The tiny
                # ACT gathers run as soon as the last K-sq partial lands; the
                # AR latency then hides under the V-tail PE work.
                gth = sb_small.tile([128, 5], f, tag="gth")
                for mt in range(CT):
                    specs = [(0, 5, 0 + mt), (5, 10, 4 + mt), (10, 14, 6 + mt)]
                    for lo, hi, col in specs:
                        nc.scalar.activation(
                            out=gth[:, 0:hi - lo],
                            in_=qk_part[:, mt, lo:hi], func=AF.Copy,
                            accum_out=sums[:, col:col + 1],
                        )
                nc.sync.dma_start(out=cc_in1[:, :], in_=sums)
                sums_g = sb_small.tile([128, 8], f, tag="sums_g")
                if skip_cc:
                    nc.sync.dma_start(out=sums_g, in_=cc_in1[:, :])
                else:
                    nc.gpsimd.collective_compute(
                        "AllReduce", ALU.add,
                        replica_groups=[list(range(N_CORES))],
                        ins=[cc_in1.ap().opt()], outs=[cc_out1.ap().opt()],
                    )
                    nc.sync.dma_start(out=sums_g, in_=cc_out1[:, :])

                # -------- Q/K affine on Pool (idle here; DVE/ACT must keep
                # draining V-tail evictions):  a = g*exp(-.5*ln(var+eps));
                # c = beta - a*mean;  then only Q' chunk 0 is critical.
                mean4 = sb_small.tile([128, 4], f, tag="mean4")
                msq4 = sb_small.tile([128, 4], f, tag="msq4")
                var4 = sb_small.tile([128, 4], f, tag="var4")
                a4 = sb_small.tile([128, 4], f, tag="a4")
                c4 = sb_small.tile([128, 4], f, tag="c4")
                # ------- V mid (PE work that hides the AR round-trip);
                # the DVE affine chain is interleaved so DVE reaches each
                # piece right as its dependencies land
                for kt in range(14, 18):
                    emit_vt(kt, borrow=(kt % 3 == 2))
                nc.vector.tensor_mul(mean4, sums_g[:, 0:4], invn4)
                nc.vector.tensor_mul(msq4, sums_g[:, 4:8], invn4)
                nc.vector.tensor_mul(var4, mean4, mean4)
                nc.vector.tensor_sub(var4, msq4, var4)
                for kt in range(18, 26):
                    emit_vt(kt, borrow=(kt % 3 == 2))

                # Ln/Exp sit here so ACT reaches them right as the AR result
                # lands, between V evictions (no head-of-line blocking); the
                # rest of the affine chain and every Q' chunk run on Pool,
                # which has been idle since its last ksq.
                nc.scalar.activation(out=var4, in_=var4, func=AF.Ln, bias=eps_t)
                nc.scalar.activation(out=a4, in_=var4, func=AF.Exp, scale=-0.5)
                nc.vector.tensor_mul(a4, vecs[:, 0:4], a4)
                nc.vector.tensor_mul(c4, a4, mean4)
                nc.vector.tensor_sub(c4, vecs[:, 6:10], c4)
                # aqk = aq*ak (folded into Q); acq = ak*cq (alpha operand)
                aqk = sb_small.tile([128, 2], f, tag="aqk")
                # acq gets a zero third column: the f32r alpha matmuls must
                # stream an even number of moving columns, so each streams
                # [acq_ci, pad] and the pad lane accumulates harmless zeros
                acq = sb_small.tile([128, 3], F32R, tag="acq")
                nc.vector.tensor_copy(
                    out=acq[:, 2:3], in_=nc.const_aps.tensor(0.0, (128, 1), F32)
                )
                nc.vector.tensor_mul(aqk, a4[:, 0:2], a4[:, 2:4])
                nc.vector.tensor_mul(acq[:, 0:2], a4[:, 2:4], c4[:, 0:2])
                nc.vector.tensor_scalar_mul(
                    qraw[:, 0, 0:512], qraw[:, 0, 0:512], aqk[:, 0:1])
                nc.scalar.activation(
                    out=qraw[:, 1, 0:512], in_=qraw[:, 1, 0:512],
                    func=AF.Copy, scale=aqk[:, 1:2])

                for kt in range(26, NKT):
                    emit_vt(kt, borrow=(kt % 3 == 2))

                # ------- alpha = K_raw^T (ak*cq); exp(alpha) into vt rows --
                pal = ps_s2.tile([128, 64], f, tag="s2", name="pal")
                for kt in range(NKT):
                    for ci in range(CT):
                        nc.tensor.matmul(
                            pal[:, 2 * kt:2 * kt + 2],
                            kraw[:, ci, kt * 128:(kt + 1) * 128],
                            acq[:, ci:ci + 2],
                            start=(ci == 0), stop=(ci == CT - 1),
                        )
                ea = sb_small.tile([128, 64], f, tag="ea")
                nc.scalar.activation(out=ea, in_=pal, func=AF.Exp)
                for kt in range(NKT):
                    nc.vector.tensor_scalar_mul(
                        vt[:, kt, :], vt[:, kt, :], ea[:, 2 * kt:2 * kt + 1]
                    )
                flush_vt(0)
                # Q' chunks 1..3 (needed only when their qb starts, ~30us+)
                for qb in range(1, 4):
                    for ci in range(CT):
                        nc.vector.tensor_scalar_mul(
                            qraw[:, ci, qb * 512:(qb + 1) * 512],
                            qraw[:, ci, qb * 512:(qb + 1) * 512],
                            aqk[:, ci:ci + 1],
                        )
                nc.scalar.activation(out=vrow[0:1, 256:512], in_=pvs[0:1, 256:512],
                                     func=AF.Copy)
                nc.sync.dma_start(out=scr_v.ap(), in_=vrow)
                # bounce row->col, AllReduce #2 (off critical path)
                nc.sync.dma_start(
                    out=sums_v[:, 0:2],
                    in_=bass.AP(tensor=scr_v, offset=0, ap=[[1, 128], [128, 2]]),
                )
                nc.sync.dma_start(
                    out=sums_v[:, 2:4],
                    in_=bass.AP(tensor=scr_v, offset=256, ap=[[1, 128], [128, 2]]),
                )
                nc.sync.dma_start(out=cc_in2[:, :], in_=sums_v)
                sums_vg = sb_small.tile([128, 4], f, tag="sums_vg")
                if skip_cc:
                    nc.sync.dma_start(out=sums_vg, in_=cc_in2[:, :])
                else:
                    nc.gpsimd.collective_compute(
                        "AllReduce", ALU.add,
                        replica_groups=[list(range(N_CORES))],
                        ins=[cc_in2.ap().opt()], outs=[cc_out2.ap().opt()],
                    )
                    nc.sync.dma_start(out=sums_vg, in_=cc_out2[:, :])

                # ------- V affine (feeds only the epilogue broadcasts) ------
                mv = sb_small.tile([128, 2], f, tag="mv")
                vv = sb_small.tile([128, 2], f, tag="vv")
                av = sb_small.tile([128, 2], f, tag="av")
                cv = sb_small.tile([128, 2], f, tag="cv")
                nc.vector.tensor_scalar_mul(mv, sums_vg[:, 0:2], 1.0 / NKV_TOT2)
                nc.vector.tensor_scalar_mul(vv, sums_vg[:, 2:4], 1.0 / NKV_TOT2)
                nc.vector.tensor_mul(av, mv, mv)
                nc.vector.tensor_sub(vv, vv, av)
                nc.scalar.activation(out=vv, in_=vv, func=AF.Ln, bias=eps_t)
                nc.scalar.activation(out=av, in_=vv, func=AF.Exp, scale=-0.5)
                nc.vector.tensor_mul(av, vecs[:, 4:6], av)
                nc.vector.tensor_mul(cv, av, mv)
                nc.vector.tensor_sub(cv, vecs[:, 10:12], cv)
                # broadcast av/cv along the free axis via DRAM bounce
                nc.sync.dma_start(
                    out=bass.AP(tensor=scr_ab, offset=0, ap=[[1, 128], [128, 2]]),
                    in_=av,
                )
                nc.sync.dma_start(
                    out=bass.AP(tensor=scr_ab, offset=256, ap=[[1, 128], [128, 2]]),
                    in_=cv,
                )
                av_b = sb_small.tile([128, 256], f, tag="av_b")
                cv_b = sb_small.tile([128, 256], f, tag="cv_b")
                nc.sync.dma_start(
                    out=av_b,
                    in_=bass.AP(tensor=scr_ab, offset=0, ap=[[0, 128], [1, 256]]),
                )
                nc.sync.dma_start(
                    out=cv_b,
                    in_=bass.AP(tensor=scr_ab, offset=256, ap=[[0, 128], [1, 256]]),
                )

                # ---------------- attention ----------------
                PAIRS = [(2 * p, 2 * p + 2) for p in range(NKT // 2)]
                for qb in range(4):
                    po = [psb1.tile([128, 258], f, tag="b1", name=f"po{qb}_{i}")
                          for i in range(4)]
                    pend = []

                    def emit_epilogues(qb=qb, po=po):
                        # all reciprocals first, then one whole chain per
                        # engine (DVE / Pool alternating) — two independent
                        # queues instead of a cross-engine ping-pong
                        rds = []
                        for qt in range(4):
                            rd = sb_small.tile([128, 1], f, tag="rd",
                                               name=f"r{qb * 4 + qt}")
                            nc.vector.reciprocal(out=rd, in_=po[qt][:, 256:257])
                            rds.append(rd)
                        for qt in range(4):
                            qg = qb * 4 + qt
                            ot = sb_out.tile([128, 256], f, tag="ot",
                                             name=f"ot{qg}")
                            # Pool cannot read PSUM: the stt (PSUM in) stays
                            # on DVE; the SBUF add alternates DVE/Pool
                            nc.vector.scalar_tensor_tensor(
                                out=ot, in0=po[qt][:, 0:256], scalar=rds[qt],
                                in1=av_b, op0=ALU.mult, op1=ALU.mult,
                            )
                            eng = nc.vector if qt % 2 == 0 else nc.gpsimd
                            eng.tensor_add(ot, ot, cv_b)
                            # final qb: split y-DMA issue over SP and ACT
                            # queues (each dma_start costs ~650ns of issue)
                            dq = nc.scalar if (qb == 3 and qt % 2 == 1) else nc.sync
                            dq.dma_start(out=y_d[qg], in_=ot)

                    def emit_out(e2, k0, k1, po=po, tail=False,
                                 emit_epilogues=emit_epilogues):
                        for kt in range(k0, k1):
                            off = (kt - k0) * 512
                            for qt in range(4):
                                nc.tensor.matmul(
                                    po[qt],
                                    e2[:, off + qt * 128:off + (qt + 1) * 128],
                                    vt[:, kt, :],
                                    start=(kt == 0), stop=(kt == NKT - 1),
                                )
                        if tail:
                            emit_epilogues()

                    for (k0, k1) in PAIRS:
                        ps_s = ps_s2.tile([128, 1024], f, tag="s2",
                                          name=f"ps{qb}_{k0}")
                        for kt in range(k0, k1):
                            off = (kt - k0) * 512
                            for ci in range(CT):
                                nc.tensor.matmul(
                                    ps_s[:, off:off + 512],
                                    kraw[:, ci, kt * 128:kt * 128 + 128],
                                    qraw[:, ci, qb * 512:(qb + 1) * 512],
                                    start=(ci == 0), stop=(ci == CT - 1),
                                )
                        e2 = sb_e.tile([128, 1024], F32R, tag="e",
                                       name=f"e{qb}_{k0}")
                        if k1 == NKT:
                            nc.scalar.activation(
                                out=e2[:, 0:512], in_=ps_s[:, 0:512],
                                func=AF.Exp, bias=negc_t,
                            )
                            nc.scalar.activation(
                                out=e2[:, 512:1024], in_=ps_s[:, 512:1024],
                                func=AF.Exp, bias=negc_t,
                            )
                        else:
                            nc.scalar.activation(
                                out=e2, in_=ps_s, func=AF.Exp, bias=negc_t,
                            )
                        if pend:
                            emit_out(*pend.pop())
                        pend.append((e2, k0, k1))
                    emit_out(*pend.pop(), tail=True)

            if reps == 1:
                body(0)
            else:
                with tc.For_i(0, reps, 1) as it:
                    body(it)
    return nc


# ---------------------------------------------------------------------------
def _prep_inputs(x, Wq, Wk, Wv, gq, betaq, gk, betak, gv, betav):
    """Build the 8 per-core input maps (all fp32, pre-laid-out)."""
    x = np.asarray(x, np.float32)
    B = x.shape[0]
    xp_full = np.zeros((B, 256, HP, HP), np.float32)
    xp_full[:, :, 1:65, 1:65] = x

    wq_t = np.ascontiguousarray(
        np.asarray(Wq, np.float32).reshape(256, CT, 128, 3, 3)
        .transpose(2, 3, 4, 1, 0)
    ).reshape(128, 9, CT, 256)
    wk_t = np.ascontiguousarray(
        np.asarray(Wk, np.float32).reshape(256, CT, 128).transpose(2, 1, 0)
    )
    wv_t = np.ascontiguousarray(
        np.asarray(Wv, np.float32).reshape(256, CT, 128).transpose(2, 1, 0)
    )
    cols = [np.asarray(v, np.float32).reshape(CT, 128).T
            for v in (gq, gk, gv, betaq, betak, betav)]
    vecs = np.concatenate(cols, axis=1).astype(np.float32)  # (128, 12)
    vecs = np.ascontiguousarray(vecs)

    in_maps = []
    for core in range(N_CORES):
        b, h = core // 2, core % 2
        # xq: the 34 padded-grid rows of my query half + halo (Q conv).
        halo = xp_full[b][:, h * 32:h * 32 + 34, :]
        xq_b = np.ascontiguousarray(
            halo.reshape(256, 34 * HP).reshape(CT, 128, 34 * HP)
            .transpose(1, 0, 2)
        )
        # xi: the 4096 interior tokens (K/V convs + colsum): query rows
        # first, then the other 32 interior rows — same order on every
        # core so all device access patterns are core-independent.
        qrows = list(range(h * 32, h * 32 + 32))
        rest = list(range(32, 64)) if h == 0 else list(range(0, 32))
        xi_b = np.ascontiguousarray(
            x[b][:, qrows + rest, :].reshape(256, NPOS)
            .reshape(CT, 128, NPOS).transpose(1, 0, 2)
        )
        in_maps.append({
            "xq": xq_b, "xi": xi_b, "wq": wq_t, "wk": wk_t, "wv": wv_t,
            "vecs": vecs,
        })
    return in_maps


_NC_CACHE = {}


def _get_nc(reps=1, skip_cc=False):
    key = (reps, skip_cc)
    if key not in _NC_CACHE:
        _NC_CACHE[key] = build_nc(reps, skip_cc)
    return _NC_CACHE[key]


def _assemble(results):
    out = np.empty((4, 256, 4096), np.float32)
    for core, r in enumerate(results):
        b, h = core // 2, core % 2
        yc = r["y"].reshape(QSH, 256)          # (q, oc)
        out[b, :, h * QSH:(h + 1) * QSH] = yc.T
    return out.reshape(4, 256, 64, 64)


def kernel(x, Wq, bq, gq, betaq, Wk, bk, gk, betak, Wv, bv, gv, betav,
           _reps=1):
    # bq/bk/bv are mathematically irrelevant: BatchNorm with batch statistics
    # removes any per-channel constant shift (including the pad-ring bias).
    in_maps = _prep_inputs(x, Wq, Wk, Wv, gq, betaq, gk, betak, gv, betav)
    nc = _get_nc(_reps)
    res = run_bass_kernel_spmd(nc, in_maps, core_ids=list(range(N_CORES)))
    return _assemble(res.results)
